# revision 32
# baseline (speedup 1.0000x reference)
"""Multi-head attention (B=2, N=2048, D=1024, H=16) on 8 TRN2 cores.

Sharding: core c -> batch b = c//4, head-group g = c%4 (4 heads each).
Each core computes its heads' attention + a partial output projection
(row-split W_output); the host sums the 4 partials per batch.

Per-core dataflow (all matmuls float32r, fp32 PSUM accumulate):
  xT [D, N] (host pre-transposed)  -> Q^T, K^T [256, N] and V [N, 256]
  per i-tile(512), per head-pair, per j-chunk(128):
     S^T[j, i] = Kh^T.T @ Qh^T   (two heads packed on PE row-groups 0-63/64-127)
     E = exp(S^T / 32)           (ScalarE, straight from PSUM)
     O^T[d, i] += [Vh | 1].T @ E (ones-augmented: row 64 = softmax denom l_i)
  normalize: copy l + O off PSUM (frees the accumulator fast), then
     reciprocal_approx_fast -> gpsimd partition_broadcast -> per-column scale
  y[i, e] = O_norm^T.T @ Wo_rows  (partial; host adds the 4 head-groups),
     streamed per i-tile so stores overlap the remaining attention.
"""

import numpy as np

B, N, D = 2, 2048, 1024
H = 16
HD = 64           # head dim
DH = 256          # per-core head columns (4 heads)
NJ = 16           # j chunks of 128 keys
NI = 4            # i tiles of 512 queries
KD = 8            # d chunks of 128 for the projections
SCALE = 1.0 / 32.0

_CACHE = {}


def _build_nc():
    import concourse.bacc as bacc
    import concourse.tile as tile
    from concourse import mybir

    F32 = mybir.dt.float32
    BF16 = mybir.dt.bfloat16
    F32R = mybir.dt.float32r
    EXP = mybir.ActivationFunctionType.Exp

    nc = bacc.Bacc("TRN2", target_bir_lowering=False, debug=False, num_devices=8)
    XT = nc.dram_tensor("xt", [D, N], BF16, kind="ExternalInput").ap()
    WQ = nc.dram_tensor("wq", [D, DH], BF16, kind="ExternalInput").ap()
    WK = nc.dram_tensor("wk", [D, DH], BF16, kind="ExternalInput").ap()
    WV = nc.dram_tensor("wv", [D, DH], BF16, kind="ExternalInput").ap()
    WO = nc.dram_tensor("wo", [DH, D], F32, kind="ExternalInput").ap()
    ONES = nc.dram_tensor("ones", [128, 68], F32, kind="ExternalInput").ap()
    Y = nc.dram_tensor("y", [N, D], F32, kind="ExternalOutput").ap()

    with tile.TileContext(nc) as tc:
        with (
            tc.tile_pool(name="per", bufs=1) as per,          # persistent SBUF
            tc.tile_pool(name="epool", bufs=7) as epool,      # exp tiles
            tc.tile_pool(name="npool", bufs=3) as npool,      # norm scratch
            tc.tile_pool(name="ypool", bufs=2) as ypool,      # y staging
            tc.tile_pool(name="ps", bufs=2, space="PSUM") as ps,    # 4 banks
            tc.tile_pool(name="po", bufs=1, space="PSUM") as po,    # 2 banks
        ):
            # ---- load phase: weights first so QT/KT matmuls chase xt ----
            wq = per.tile([128, KD, DH], BF16, tag="wq")
            wk = per.tile([128, KD, DH], BF16, tag="wk")
            wv = per.tile([128, KD, DH], BF16, tag="wv")
            xt = [per.tile([128, N], BF16, tag=f"xt{k}", name=f"xt{k}") for k in range(KD)]
            nc.sync.dma_start(wq[:], WQ.rearrange("(k p) e -> p k e", p=128))
            nc.sync.dma_start(wk[:], WK.rearrange("(k p) e -> p k e", p=128))
            for k in range(KD):
                nc.sync.dma_start(xt[k][:], XT[k * 128 : (k + 1) * 128, :])
            nc.sync.dma_start(wv[:], WV.rearrange("(k p) e -> p k e", p=128))
            ones = per.tile([128, 68], F32R, tag="ones")
            nc.sync.dma_start(ones[:], ONES.bitcast(F32R))
            wo = per.tile([128, 2, D], F32R, tag="wo")
            nc.sync.dma_start(wo[:], WO.rearrange("(k p) e -> p k e", p=128).bitcast(F32R))

            QT = [per.tile([128, N], F32R, tag=f"qt{p}", name=f"qt{p}") for p in range(2)]
            KT = [per.tile([128, N], F32R, tag=f"kt{p}", name=f"kt{p}") for p in range(2)]

            def proj_acc(dst, w, p, t4, tag):
                c0 = t4 * 512
                acc = ps.tile([128, 512], F32, tag=tag, name=f"pa{dst is QT}{p}{t4}")
                for k in range(KD):
                    nc.tensor.matmul(
                        acc[:],
                        w[:, k, p * 128 : (p + 1) * 128],
                        xt[k][:, c0 : c0 + 512],
                        start=(k == 0),
                        stop=(k == KD - 1),
                    )
                nc.vector.tensor_copy(dst[p][:, c0 : c0 + 512], acc[:])

            # startup: six projection accumulators chase the xt DMAs
            # concurrently (dual accs per wide "s" slot + the 2 qkv slots)
            def proj_acc2(spec_a, spec_b):
                accw = ps.tile([128, 1024], F32, tag="s", name="pw")
                for half, (dst, w, p, t4) in enumerate((spec_a, spec_b)):
                    c0 = t4 * 512
                    for k in range(KD):
                        nc.tensor.matmul(
                            accw[:, half * 512 : (half + 1) * 512],
                            w[:, k, p * 128 : (p + 1) * 128],
                            xt[k][:, c0 : c0 + 512],
                            start=(k == 0),
                            stop=(k == KD - 1),
                        )
                for half, (dst, w, p, t4) in enumerate((spec_a, spec_b)):
                    nc.scalar.copy(
                        dst[p][:, t4 * 512 : (t4 + 1) * 512],
                        accw[:, half * 512 : (half + 1) * 512],
                    )

            proj_acc2((KT, wk, 0, 0), (QT, wq, 0, 0))
            proj_acc2((KT, wk, 0, 1), (KT, wk, 0, 2))
            proj_acc(KT, wk, 0, 3, "qkv")
            proj_acc(QT, wq, 0, 1, "qkv")

            # ---- deferred PE work: V columns + pair-1 projections, drained
            # ---- as filler inside the first attention i-tile loops ----
            Vaug = [per.tile([128, 260], F32R, tag=f"va{j}", name=f"va{j}") for j in range(NJ)]

            def build_v(j):
                acc = ps.tile([128, 256], F32, tag="qkv", name=f"vps{j}")
                for k in range(KD):
                    nc.tensor.matmul(
                        acc[:],
                        xt[k][:, j * 128 : (j + 1) * 128],
                        wv[:, k, :],
                        start=(k == 0),
                        stop=(k == KD - 1),
                    )
                va = Vaug[j][:].rearrange("p (h s) -> p h s", h=4)
                nc.vector.tensor_copy(
                    va[:, :, 0:64], acc[:].rearrange("p (h s) -> p h s", h=4)
                )
                nc.vector.tensor_copy(
                    va[:, :, 64:65],
                    ones[:, 64:68].rearrange("p (h s) -> p h s", s=1),
                )

            thunks = [(build_v, (j,)) for j in range(0, 12)]
            thunks += [(proj_acc, (KT, wk, 1, 0, "qkv")), (proj_acc, (QT, wq, 1, 0, "qkv"))]
            thunks += [(build_v, (j,)) for j in range(12, NJ)]
            for t4 in range(1, 4):
                thunks.append((proj_acc, (KT, wk, 1, t4, "qkv")))
            thunks.append((proj_acc, (QT, wq, 0, 2, "qkv")))
            thunks.append((proj_acc, (QT, wq, 0, 3, "qkv")))
            for t4 in range(1, 4):
                thunks.append((proj_acc, (QT, wq, 1, t4, "qkv")))
            thunks.reverse()  # pop() drains in order

            # ---- attention (pairs interleaved per i-tile) + y streaming ----
            ON = [per.tile([128, N], F32R, tag=f"on{p}", name=f"on{p}") for p in range(2)]

            def y_unit(ic, last=False):
                ysb = ypool.tile([128, 1024], F32, tag="y", name=f"y{ic}")
                for et in range(2):
                    ptag = "s" if (last and et == 1) else "qkv"
                    py = ps.tile([128, 512], F32, tag=ptag, name=f"py{ic}{et}")
                    for p in range(2):
                        nc.tensor.matmul(
                            py[:],
                            ON[p][:, ic * 128 : (ic + 1) * 128],
                            wo[:, p, et * 512 : (et + 1) * 512],
                            start=(p == 0),
                            stop=(p == 1),
                        )
                    if last and et == 0:
                        nc.scalar.copy(ysb[:, et * 512 : (et + 1) * 512], py[:])
                    else:
                        nc.vector.tensor_copy(ysb[:, et * 512 : (et + 1) * 512], py[:])
                nc.sync.dma_start(Y[ic * 128 : (ic + 1) * 128, :], ysb[:])

            def emit_y(it):
                for icc in range(4):
                    y_unit(it * 4 + icc, last=(it == NI - 1))

            for it in range(NI):
                i0 = it * 512
                if it >= 1:
                    for icc in range(4):
                        thunks.insert(0, (y_unit, ((it - 1) * 4 + icc,)))
                for p in range(2):
                    O = po.tile([65, 1024], F32, tag="o", name=f"o{p}{it}")
                    def pv(e_prev, jp):
                        for hh in range(2):
                            h = 2 * p + hh
                            nc.tensor.matmul(
                                O[:, hh * 512 : (hh + 1) * 512],
                                Vaug[jp][:, 65 * h : 65 * h + 65],
                                e_prev[:, hh * 512 : (hh + 1) * 512],
                                start=(jp == 0),
                                stop=(jp == NJ - 1),
                            )

                    LAG = 5
                    epend = []
                    for j in range(NJ):
                        s = ps.tile([128, 1024], F32, tag="s", name=f"s{p}{it}{j}")
                        e = epool.tile([128, 1024], F32R, tag="e", name=f"e{p}{it}{j}")
                        for hh in range(2):  # packed head pair on PE rows
                            pr = slice(hh * 64, hh * 64 + 64)
                            nc.tensor.matmul(
                                s[:, hh * 512 : (hh + 1) * 512],
                                KT[p][pr, j * 128 : (j + 1) * 128],
                                QT[p][pr, i0 : i0 + 512],
                                start=True,
                                stop=True,
                            )
                        nc.scalar.activation(e[:], s[:], EXP, scale=SCALE)
                        if thunks:
                            fn, a = thunks.pop()
                            fn(*a)
                        if thunks and (j % 2 == 1):
                            fn, a = thunks.pop()
                            fn(*a)
                        epend.append((e, j))
                        if len(epend) > LAG:  # lag keeps PE stream ahead of O stalls
                            pv(*epend.pop(0))
                    for ej in epend:
                        pv(*ej)
                    # normalization: drain O with 2 copies/head, then scale
                    # off-PSUM.  NB: custom-DVE recip needs base-partition-0 input.
                    ou = []
                    rv = []
                    tailn = it == NI - 1 and p == 1
                    for hh in range(2):
                        co = hh * 512
                        linv = npool.tile([1, 512], F32, tag="linv", name=f"li{p}{it}{hh}")
                        nc.vector.tensor_copy(linv[:], O[64:65, co : co + 512])
                        o_sb = npool.tile([64, 512], F32, tag="osb", name=f"ou{p}{it}{hh}")
                        if tailn:  # ScalarE is idle at the tail; halve the DVE chain
                            nc.scalar.copy(o_sb[:], O[0:64, co : co + 512])
                        else:
                            nc.vector.tensor_copy(o_sb[:], O[0:64, co : co + 512])
                        rinv = npool.tile([1, 512], F32, tag="rinv", name=f"ri{p}{it}{hh}")
                        nc.vector.reciprocal_approx_fast(rinv[:], linv[:])
                        ou.append(o_sb)
                        rv.append(rinv)
                    for hh in range(2):
                        r64 = npool.tile([64, 512], F32, tag="r64", name=f"r{p}{it}{hh}")
                        nc.gpsimd.partition_broadcast(r64[:], rv[hh][:])
                        nc.vector.tensor_mul(
                            ON[p][hh * 64 : hh * 64 + 64, i0 : i0 + 512],
                            ou[hh][:],
                            r64[:],
                        )

            while thunks:
                fn, a = thunks.pop()
                fn(*a)
            emit_y(NI - 1)

    nc.compile()
    return nc


def _get_nc():
    if "nc" not in _CACHE:
        _CACHE["nc"] = _build_nc()
    return _CACHE["nc"]


def make_in_maps(x, W_query, W_key, W_value, W_output):
    import ml_dtypes

    bf16 = ml_dtypes.bfloat16
    ones = np.ones((128, 68), np.float32)
    in_maps = []
    for c in range(8):
        b, g = c // 4, c % 4
        in_maps.append(
            {
                "xt": np.ascontiguousarray(x[b].T).astype(bf16),
                "wq": np.ascontiguousarray(W_query[:, g * DH : (g + 1) * DH]).astype(bf16),
                "wk": np.ascontiguousarray(W_key[:, g * DH : (g + 1) * DH]).astype(bf16),
                "wv": np.ascontiguousarray(W_value[:, g * DH : (g + 1) * DH]).astype(bf16),
                "wo": np.ascontiguousarray(W_output[g * DH : (g + 1) * DH, :]),
                "ones": ones,
            }
        )
    return in_maps


def run(x, W_query, W_key, W_value, W_output, trace=False):
    from concourse.bass_utils import run_bass_kernel_spmd

    nc = _get_nc()
    in_maps = make_in_maps(x, W_query, W_key, W_value, W_output)
    res = run_bass_kernel_spmd(nc, in_maps, list(range(8)), trace=trace)
    parts = [res.results[c]["y"] for c in range(8)]
    out = np.stack(
        [parts[0] + parts[1] + parts[2] + parts[3],
         parts[4] + parts[5] + parts[6] + parts[7]]
    ).astype(np.float32)
    return out, res


def kernel(x, W_query, W_key, W_value, W_output):
    x = np.asarray(x, dtype=np.float32)
    W_query = np.asarray(W_query, dtype=np.float32)
    W_key = np.asarray(W_key, dtype=np.float32)
    W_value = np.asarray(W_value, dtype=np.float32)
    W_output = np.asarray(W_output, dtype=np.float32)
    out, _ = run(x, W_query, W_key, W_value, W_output, trace=False)
    return out


# revision 33
# speedup vs baseline: 1.0080x; 1.0080x over previous
"""Multi-head attention (B=2, N=2048, D=1024, H=16) on 8 TRN2 cores.

Sharding: core c -> batch b = c//4, head-group g = c%4 (4 heads each).
Each core computes its heads' attention + a partial output projection
(row-split W_output); the host sums the 4 partials per batch.

Per-core dataflow (all matmuls float32r, fp32 PSUM accumulate):
  xT [D, N] (host pre-transposed)  -> Q^T, K^T [256, N] and V [N, 256]
  per i-tile(512), per head-pair, per j-chunk(128):
     S^T[j, i] = Kh^T.T @ Qh^T   (two heads packed on PE row-groups 0-63/64-127)
     E = exp(S^T / 32)           (ScalarE, straight from PSUM)
     O^T[d, i] += [Vh | 1].T @ E (ones-augmented: row 64 = softmax denom l_i)
  normalize: copy l + O off PSUM (frees the accumulator fast), then
     reciprocal_approx_fast -> gpsimd partition_broadcast -> per-column scale
  y[i, e] = O_norm^T.T @ Wo_rows  (partial; host adds the 4 head-groups),
     streamed per i-tile so stores overlap the remaining attention.
"""

import numpy as np

B, N, D = 2, 2048, 1024
H = 16
HD = 64           # head dim
DH = 256          # per-core head columns (4 heads)
NJ = 16           # j chunks of 128 keys
NI = 4            # i tiles of 512 queries
KD = 8            # d chunks of 128 for the projections
SCALE = 1.0 / 32.0

_CACHE = {}


def _build_nc():
    import concourse.bacc as bacc
    import concourse.tile as tile
    from concourse import mybir

    F32 = mybir.dt.float32
    BF16 = mybir.dt.bfloat16
    F32R = mybir.dt.float32r
    EXP = mybir.ActivationFunctionType.Exp

    nc = bacc.Bacc("TRN2", target_bir_lowering=False, debug=False, num_devices=8)
    XT = nc.dram_tensor("xt", [D, N], BF16, kind="ExternalInput").ap()
    WQ = nc.dram_tensor("wq", [D, DH], BF16, kind="ExternalInput").ap()
    WK = nc.dram_tensor("wk", [D, DH], BF16, kind="ExternalInput").ap()
    WV = nc.dram_tensor("wv", [D, DH], BF16, kind="ExternalInput").ap()
    WO = nc.dram_tensor("wo", [DH, D], F32, kind="ExternalInput").ap()
    ONES = nc.dram_tensor("ones", [128, 68], F32, kind="ExternalInput").ap()
    Y = nc.dram_tensor("y", [N, D], F32, kind="ExternalOutput").ap()

    with tile.TileContext(nc) as tc:
        with (
            tc.tile_pool(name="per", bufs=1) as per,          # persistent SBUF
            tc.tile_pool(name="epool", bufs=7) as epool,      # exp tiles
            tc.tile_pool(name="npool", bufs=3) as npool,      # norm scratch
            tc.tile_pool(name="ypool", bufs=2) as ypool,      # y staging
            tc.tile_pool(name="ps", bufs=2, space="PSUM") as ps,    # 4 banks
            tc.tile_pool(name="po", bufs=1, space="PSUM") as po,    # 2 banks
        ):
            # ---- load phase: weights first so QT/KT matmuls chase xt ----
            wq = per.tile([128, KD, DH], BF16, tag="wq")
            wk = per.tile([128, KD, DH], BF16, tag="wk")
            wv = per.tile([128, KD, DH], BF16, tag="wv")
            xt = [per.tile([128, N], BF16, tag=f"xt{k}", name=f"xt{k}") for k in range(KD)]
            nc.sync.dma_start(wq[:], WQ.rearrange("(k p) e -> p k e", p=128))
            nc.sync.dma_start(wk[:], WK.rearrange("(k p) e -> p k e", p=128))
            for k in range(KD):
                nc.sync.dma_start(xt[k][:], XT[k * 128 : (k + 1) * 128, :])
            nc.sync.dma_start(wv[:], WV.rearrange("(k p) e -> p k e", p=128))
            ones = per.tile([128, 68], F32R, tag="ones")
            nc.sync.dma_start(ones[:], ONES.bitcast(F32R))
            wo = per.tile([128, 2, D], F32R, tag="wo")
            nc.sync.dma_start(wo[:], WO.rearrange("(k p) e -> p k e", p=128).bitcast(F32R))

            QT = [per.tile([128, N], F32R, tag=f"qt{p}", name=f"qt{p}") for p in range(2)]
            KT = [per.tile([128, N], F32R, tag=f"kt{p}", name=f"kt{p}") for p in range(2)]

            def proj_acc(dst, w, p, t4, tag):
                c0 = t4 * 512
                acc = ps.tile([128, 512], F32, tag=tag, name=f"pa{dst is QT}{p}{t4}")
                for k in range(KD):
                    nc.tensor.matmul(
                        acc[:],
                        w[:, k, p * 128 : (p + 1) * 128],
                        xt[k][:, c0 : c0 + 512],
                        start=(k == 0),
                        stop=(k == KD - 1),
                    )
                nc.vector.tensor_copy(dst[p][:, c0 : c0 + 512], acc[:])

            # startup: six projection accumulators chase the xt DMAs
            # concurrently (dual accs per wide "s" slot + the 2 qkv slots)
            def proj_acc2(spec_a, spec_b, copy_eng):
                accw = ps.tile([128, 1024], F32, tag="s", name="pw")
                for k in range(KD):  # halves interleaved so both chase the DMAs
                    for half, (dst, w, p, t4) in enumerate((spec_a, spec_b)):
                        c0 = t4 * 512
                        nc.tensor.matmul(
                            accw[:, half * 512 : (half + 1) * 512],
                            w[:, k, p * 128 : (p + 1) * 128],
                            xt[k][:, c0 : c0 + 512],
                            start=(k == 0),
                            stop=(k == KD - 1),
                        )
                for half, (dst, w, p, t4) in enumerate((spec_a, spec_b)):
                    copy_eng(
                        dst[p][:, t4 * 512 : (t4 + 1) * 512],
                        accw[:, half * 512 : (half + 1) * 512],
                    )

            proj_acc2((KT, wk, 0, 0), (QT, wq, 0, 0), nc.vector.tensor_copy)
            proj_acc2((KT, wk, 0, 1), (KT, wk, 0, 2), nc.scalar.copy)
            proj_acc(KT, wk, 0, 3, "qkv")
            proj_acc(QT, wq, 0, 1, "qkv")

            # ---- deferred PE work: V columns + pair-1 projections, drained
            # ---- as filler inside the first attention i-tile loops ----
            Vaug = [per.tile([128, 260], F32R, tag=f"va{j}", name=f"va{j}") for j in range(NJ)]

            def build_v(j):
                acc = ps.tile([128, 256], F32, tag="qkv", name=f"vps{j}")
                for k in range(KD):
                    nc.tensor.matmul(
                        acc[:],
                        xt[k][:, j * 128 : (j + 1) * 128],
                        wv[:, k, :],
                        start=(k == 0),
                        stop=(k == KD - 1),
                    )
                va = Vaug[j][:].rearrange("p (h s) -> p h s", h=4)
                nc.vector.tensor_copy(
                    va[:, :, 0:64], acc[:].rearrange("p (h s) -> p h s", h=4)
                )
                nc.vector.tensor_copy(
                    va[:, :, 64:65],
                    ones[:, 64:68].rearrange("p (h s) -> p h s", s=1),
                )

            thunks = [(build_v, (j,)) for j in range(0, 12)]
            thunks += [(proj_acc, (KT, wk, 1, 0, "qkv")), (proj_acc, (QT, wq, 1, 0, "qkv"))]
            thunks += [(build_v, (j,)) for j in range(12, NJ)]
            for t4 in range(1, 4):
                thunks.append((proj_acc, (KT, wk, 1, t4, "qkv")))
            thunks.append((proj_acc, (QT, wq, 0, 2, "qkv")))
            thunks.append((proj_acc, (QT, wq, 0, 3, "qkv")))
            for t4 in range(1, 4):
                thunks.append((proj_acc, (QT, wq, 1, t4, "qkv")))
            thunks.reverse()  # pop() drains in order

            # ---- attention (pairs interleaved per i-tile) + y streaming ----
            ON = [per.tile([128, N], F32R, tag=f"on{p}", name=f"on{p}") for p in range(2)]

            def y_unit(ic, last=False):
                ysb = ypool.tile([128, 1024], F32, tag="y", name=f"y{ic}")
                for et in range(2):
                    ptag = "s" if (last and et == 1) else "qkv"
                    py = ps.tile([128, 512], F32, tag=ptag, name=f"py{ic}{et}")
                    for p in range(2):
                        nc.tensor.matmul(
                            py[:],
                            ON[p][:, ic * 128 : (ic + 1) * 128],
                            wo[:, p, et * 512 : (et + 1) * 512],
                            start=(p == 0),
                            stop=(p == 1),
                        )
                    if last and et == 0:
                        nc.scalar.copy(ysb[:, et * 512 : (et + 1) * 512], py[:])
                    else:
                        nc.vector.tensor_copy(ysb[:, et * 512 : (et + 1) * 512], py[:])
                nc.sync.dma_start(Y[ic * 128 : (ic + 1) * 128, :], ysb[:])

            def emit_y(it):
                for icc in range(4):
                    y_unit(it * 4 + icc, last=(it == NI - 1))

            for it in range(NI):
                i0 = it * 512
                if it >= 1:
                    for icc in range(4):
                        thunks.insert(0, (y_unit, ((it - 1) * 4 + icc,)))
                for p in range(2):
                    O = po.tile([65, 1024], F32, tag="o", name=f"o{p}{it}")
                    def pv(e_prev, jp):
                        for hh in range(2):
                            h = 2 * p + hh
                            nc.tensor.matmul(
                                O[:, hh * 512 : (hh + 1) * 512],
                                Vaug[jp][:, 65 * h : 65 * h + 65],
                                e_prev[:, hh * 512 : (hh + 1) * 512],
                                start=(jp == 0),
                                stop=(jp == NJ - 1),
                            )

                    LAG = 5
                    epend = []
                    for j in range(NJ):
                        s = ps.tile([128, 1024], F32, tag="s", name=f"s{p}{it}{j}")
                        e = epool.tile([128, 1024], F32R, tag="e", name=f"e{p}{it}{j}")
                        for hh in range(2):  # packed head pair on PE rows
                            pr = slice(hh * 64, hh * 64 + 64)
                            nc.tensor.matmul(
                                s[:, hh * 512 : (hh + 1) * 512],
                                KT[p][pr, j * 128 : (j + 1) * 128],
                                QT[p][pr, i0 : i0 + 512],
                                start=True,
                                stop=True,
                            )
                        nc.scalar.activation(e[:], s[:], EXP, scale=SCALE)
                        if thunks:
                            fn, a = thunks.pop()
                            fn(*a)
                        if thunks and (j % 2 == 1):
                            fn, a = thunks.pop()
                            fn(*a)
                        epend.append((e, j))
                        if len(epend) > LAG:  # lag keeps PE stream ahead of O stalls
                            pv(*epend.pop(0))
                    for ej in epend:
                        pv(*ej)
                    # normalization: drain O with 2 copies/head, then scale
                    # off-PSUM.  NB: custom-DVE recip needs base-partition-0 input.
                    ou = []
                    rv = []
                    tailn = it == NI - 1 and p == 1
                    for hh in range(2):
                        co = hh * 512
                        linv = npool.tile([1, 512], F32, tag="linv", name=f"li{p}{it}{hh}")
                        nc.vector.tensor_copy(linv[:], O[64:65, co : co + 512])
                        o_sb = npool.tile([64, 512], F32, tag="osb", name=f"ou{p}{it}{hh}")
                        if tailn:  # ScalarE is idle at the tail; halve the DVE chain
                            nc.scalar.copy(o_sb[:], O[0:64, co : co + 512])
                        else:
                            nc.vector.tensor_copy(o_sb[:], O[0:64, co : co + 512])
                        rinv = npool.tile([1, 512], F32, tag="rinv", name=f"ri{p}{it}{hh}")
                        nc.vector.reciprocal_approx_fast(rinv[:], linv[:])
                        ou.append(o_sb)
                        rv.append(rinv)
                    for hh in range(2):
                        r64 = npool.tile([64, 512], F32, tag="r64", name=f"r{p}{it}{hh}")
                        nc.gpsimd.partition_broadcast(r64[:], rv[hh][:])
                        nc.vector.tensor_mul(
                            ON[p][hh * 64 : hh * 64 + 64, i0 : i0 + 512],
                            ou[hh][:],
                            r64[:],
                        )

            while thunks:
                fn, a = thunks.pop()
                fn(*a)
            emit_y(NI - 1)

    nc.compile()
    return nc


def _get_nc():
    if "nc" not in _CACHE:
        _CACHE["nc"] = _build_nc()
    return _CACHE["nc"]


def make_in_maps(x, W_query, W_key, W_value, W_output):
    import ml_dtypes

    bf16 = ml_dtypes.bfloat16
    ones = np.ones((128, 68), np.float32)
    in_maps = []
    for c in range(8):
        b, g = c // 4, c % 4
        in_maps.append(
            {
                "xt": np.ascontiguousarray(x[b].T).astype(bf16),
                "wq": np.ascontiguousarray(W_query[:, g * DH : (g + 1) * DH]).astype(bf16),
                "wk": np.ascontiguousarray(W_key[:, g * DH : (g + 1) * DH]).astype(bf16),
                "wv": np.ascontiguousarray(W_value[:, g * DH : (g + 1) * DH]).astype(bf16),
                "wo": np.ascontiguousarray(W_output[g * DH : (g + 1) * DH, :]),
                "ones": ones,
            }
        )
    return in_maps


def run(x, W_query, W_key, W_value, W_output, trace=False):
    from concourse.bass_utils import run_bass_kernel_spmd

    nc = _get_nc()
    in_maps = make_in_maps(x, W_query, W_key, W_value, W_output)
    res = run_bass_kernel_spmd(nc, in_maps, list(range(8)), trace=trace)
    parts = [res.results[c]["y"] for c in range(8)]
    out = np.stack(
        [parts[0] + parts[1] + parts[2] + parts[3],
         parts[4] + parts[5] + parts[6] + parts[7]]
    ).astype(np.float32)
    return out, res


def kernel(x, W_query, W_key, W_value, W_output):
    x = np.asarray(x, dtype=np.float32)
    W_query = np.asarray(W_query, dtype=np.float32)
    W_key = np.asarray(W_key, dtype=np.float32)
    W_value = np.asarray(W_value, dtype=np.float32)
    W_output = np.asarray(W_output, dtype=np.float32)
    out, _ = run(x, W_query, W_key, W_value, W_output, trace=False)
    return out
